# revision 59
# baseline (speedup 1.0000x reference)
"""Trainium2 Bass kernel for the masked-softmax attention module.

Computation (per batch row b):
    m      = lrelu(values[b] @ Wv.T + bv) + lrelu(query[b] @ Wq.T + bq)   [L, A]
    logit  = lrelu(tanh(m) @ Wo.T + bo)                                    [L]
    logit  = where(mask[b] == 0, -1e-9, logit)
    prob   = softmax(logit)
    out[b] = prob @ values[b]                                              [D]

Sparsity: positions with mask==0 get logit = -1e-9, so their softmax
weight is exactly exp(-1e-9) == 1.0f regardless of the expensive
pipeline. Host-side we PERMUTE each batch's L dim so mask==1 positions
come first (n1 of them), and only compute the m/tanh/Wo pipeline for
the first N >= max_b(n1) positions. Logits at [n1, N) are masked to
-1e-9 (exactly as the reference masks them) and [N, L) are memset to 0
(exp(0) == exp(-1e-9) == 1.0f). The softmax + out GEMM then run over
the full permuted L — numerically identical to the dense reference.

Main GEMM and q-projection run in fp8 (e4m3): values/query cast
directly (absmax ~5.4 << 240), Wv/Wq pre-scaled by 2^10 so their
entries are normal-range; the 2^-10 unscale is folded into the lrelu
ACT's scale input (exact, power of two). DoubleRow perf mode processes
2 k-tiles per matmul.

The per-batch softmax epilogue (exp/recip/transpose/out-GEMM) is
software-pipelined into the NEXT batch's first chunk so the in-order
PE never stalls on the softmax chain at batch boundaries.

Sharding: data-parallel over batch, 4 batches per core on 8 NeuronCores.
"""

import os
import sys

if "/opt/trn_rl_repo" not in sys.path:
    sys.path.insert(0, "/opt/trn_rl_repo")

import numpy as np
import ml_dtypes

from contextlib import ExitStack

import concourse.bass as bass
import concourse.tile as tile
from concourse import bacc, mybir
from concourse import bass_utils

BF = ml_dtypes.bfloat16
E4 = ml_dtypes.float8_e4m3
F32d = mybir.dt.float32
BF16d = mybir.dt.bfloat16
FP8d = mybir.dt.float8e4
AF = mybir.ActivationFunctionType
DR = mybir.MatmulPerfMode.DoubleRow

NCORES = 8
B, L, D, A = 32, 1024, 2048, 2048
BL = B // NCORES          # batches per core
KD = D // 128             # d tiles
KA = A // 128             # a tiles
JL = L // 128             # l tiles
ALPHA = 0.01              # leaky relu slope
W_SCALE = 1024.0          # host premultiplier on Wv/Wq for fp8 dynamic range
EPI_A = 1                 # j-slot where prev batch's softmax ops are emitted
EPI_B = 6                 # j-slot where prev batch's transpose+out GEMM goes


def build_graph(N, mm="fp8", tail_warm=False):
    """Build the per-core Bass graph (identical on all cores).

    N: padded count of computed positions per batch (even).
    mm: "fp8" (DoubleRow e4m3 main GEMM + qproj) or "bf16".
    tail_warm: keep-warm dummies + ACT-table pre-swap before the last
    epilogue.
    """
    nc = bacc.Bacc("TRN2", target_bir_lowering=False, debug=False)
    NL1 = N // 2
    chunks = [(0, NL1), (NL1, NL1)]
    fp8 = mm == "fp8"
    vdt = FP8d if fp8 else BF16d

    NT = (N + 127) // 128  # l-tiles needed by the gathered out-GEMM

    # layouts are chosen so every DMA slice is one LINEAR run in dram
    # (scattered per-partition rows run ~4x slower on the DGE rings)
    vt = nc.dram_tensor("vt", [BL, KD // 2, 128, 2, N], vdt, kind="ExternalInput")
    vn = nc.dram_tensor("vn", [BL, NT, 128, D], BF16d, kind="ExternalInput")
    vshl = nc.dram_tensor("vshl", [BL, 2, D], BF16d, kind="ExternalInput")
    wv = nc.dram_tensor("wv", [KA, 128, KD, 128], vdt, kind="ExternalInput")
    wq = nc.dram_tensor("wq", [KA, 128, KD, 128], vdt, kind="ExternalInput")
    qt = nc.dram_tensor("qt", [128, KD, BL], vdt, kind="ExternalInput")
    wo = nc.dram_tensor("wo", [128, KA], BF16d, kind="ExternalInput")
    bvt = nc.dram_tensor("bvt", [128, KA], F32d, kind="ExternalInput")
    bqt = nc.dram_tensor("bqt", [128, KA], F32d, kind="ExternalInput")
    bo = nc.dram_tensor("bo", [1, 1], F32d, kind="ExternalInput")
    mf = nc.dram_tensor("mf", [BL, N], F32d, kind="ExternalInput")
    id4d = nc.dram_tensor("id4", [JL, JL], F32d, kind="ExternalInput")
    out = nc.dram_tensor("out", [BL, D], F32d, kind="ExternalOutput")

    lr_scale = 1.0 / W_SCALE if fp8 else 1.0

    with tile.TileContext(nc) as tc, ExitStack() as ctx:
        const = ctx.enter_context(tc.tile_pool(name="const", bufs=1))
        wvp = ctx.enter_context(tc.tile_pool(name="wvp", bufs=1))

        # One FIFO HWDGE ring (sync) carries all bulk loads in exact
        # need-order (the 16 shared DMA engines serve one queue at a time,
        # so a bulk transfer on another ring would starve the weight
        # stream); the scalar ring carries only small constants.
        qts_sb = const.tile([128, KD, BL], vdt)
        nc.sync.dma_start(qts_sb[:], qt.ap()[:])
        bq_sb = const.tile([128, KA], F32d)
        nc.sync.dma_start(bq_sb[:], bqt.ap()[:])
        id8 = const.tile([JL, JL], F32d)
        nc.scalar.dma_start(id8[:], id4d.ap()[:])
        ones8 = const.tile([JL, 1], F32d)
        nc.vector.memset(ones8[:], 1.0)
        ones2b = const.tile([2, 1], BF16d)
        nc.vector.memset(ones2b[:], 1.0)
        qp_sb = const.tile([128, KA, BL], F32d)
        # allocated here, DMA'd after the first vt chunk (see main loop)
        wo_sb = const.tile([128, KA], BF16d)
        bv_sb = const.tile([128, KA], F32d)
        bo_sb = const.tile([1, 1], F32d)

        # wv is laid out a-tile-major: GEMM group j only needs its own chunk,
        # so chunks stream just-in-time, interleaved with wq below.
        wv_sb = wvp.tile([128, KA, KD, 128], vdt)
        wv_loaded = set()

        def fetch_wv(j):
            if j < KA and j not in wv_loaded:
                nc.sync.dma_start(wv_sb[:, j, :, :], wv.ap()[j, :, :, :])
                wv_loaded.add(j)

        # q-projection is interleaved into batch 0 / chunk 0 of the main loop
        # (one group per GEMM group) so its wq DMA demand spreads out and the
        # PE never sits idle waiting for the projection phase.
        wqp = ctx.enter_context(tc.tile_pool(name="wqp", bufs=6 if NT <= 5 else 4))
        psqp = ctx.enter_context(tc.tile_pool(name="psq", bufs=1, space="PSUM"))
        wq_tiles = {}

        def fetch_wq(t):
            if t < KA and t not in wq_tiles:
                wq_t = wqp.tile([128, KD, 128], vdt)
                nc.sync.dma_start(wq_t[:], wq.ap()[t, :, :, :])
                wq_tiles[t] = wq_t

        def qproj_group(t):
            wq_t = wq_tiles.pop(t)
            psq = psqp.tile([128, BL], F32d)
            if fp8:
                for q in range(KD // 2):
                    nc.tensor.matmul(
                        psq[:], lhsT=wq_t[:, 2 * q : 2 * q + 2, :],
                        rhs=qts_sb[:, 2 * q : 2 * q + 2, :],
                        start=(q == 0), stop=(q == KD // 2 - 1), perf_mode=DR,
                    )
            else:
                for k in range(KD):
                    nc.tensor.matmul(
                        psq[:], lhsT=wq_t[:, k, :], rhs=qts_sb[:, k, :],
                        start=(k == 0), stop=(k == KD - 1),
                    )
            nc.scalar.activation(
                qp_sb[:, t, :], psq[:], AF.Lrelu, bias=bq_sb[:, t : t + 1],
                scale=lr_scale, alpha=ALPHA,
            )

        # PE warmup: dummy matmuls on zeroed tiles while the first DMAs land,
        # so the HAM clock gate is released before real work starts.
        wu_l = const.tile([128, 128], BF16d)
        nc.vector.memset(wu_l[:], 0.0)
        wu_ps = psqp.tile([128, 128], F32d, tag="psq")
        for i in range(32):
            nc.tensor.matmul(wu_ps[:], lhsT=wu_l[:], rhs=wu_l[:], start=(i == 0), stop=(i == 31))

        # ---- main loop ----
        vtp = ctx.enter_context(tc.tile_pool(name="vtp", bufs=2))
        vnp = ctx.enter_context(tc.tile_pool(name="vnp", bufs=2))
        s1p = ctx.enter_context(tc.tile_pool(name="s1p", bufs=2))
        thp = ctx.enter_context(tc.tile_pool(name="thp", bufs=4))
        smp = ctx.enter_context(tc.tile_pool(name="smp", bufs=1))
        outp = ctx.enter_context(tc.tile_pool(name="outp", bufs=1))
        psm = ctx.enter_context(tc.tile_pool(name="psm", bufs=3, space="PSUM"))
        psl = ctx.enter_context(tc.tile_pool(name="psl", bufs=1, space="PSUM"))
        pst = ctx.enter_context(tc.tile_pool(name="pst", bufs=1, space="PSUM"))
        pso = ctx.enter_context(tc.tile_pool(name="pso", bufs=2, space="PSUM"))

        state = {}  # b -> (logit_sb, [vn tiles], vshl tile)

        def epi_softmax(pb):
            """Softmax chain for batch pb (ACT/DVE/DMA engines only).

            Reshape logits [1, L] -> [JL, 128] first (tiny DMA), then exp on
            8 partitions (8x faster than a 1-partition exp) with per-
            partition accumulators; Z is reduced by a tiny ones-matmul.

            The out GEMM uses the identity (gathered form):
              out*Z = sum_{i<NT*128} (e^{logit_i} - 1) v_i + sum_all v
            (positions >= n1 have logit 0 -> weight 0), so only NT l-tiles
            of values are streamed; sum_all v enters as a bf16 hi+lo pair."""
            logit_pb = state[pb][0]
            l8 = smp.tile([JL, 128], F32d, tag="l8", bufs=2)
            nc.sync.dma_start(l8[:], logit_pb[:])
            p8 = smp.tile([JL, 128], F32d, tag="p8", bufs=2)
            zpart = smp.tile([JL, 1], F32d, tag="zp", bufs=2)
            nc.scalar.activation(p8[:], l8[:], AF.Exp, accum_out=zpart[:])
            pm1 = smp.tile([NT, 128], F32d, tag="pm1", bufs=2)
            nc.vector.tensor_scalar(
                pm1[:], p8[0:NT, :], -1.0, None, mybir.AluOpType.add
            )
            return zpart, pm1

        def epi_out(pb, zpart, pm1):
            """Transpose + out GEMM for batch pb (PE heavy)."""
            _, vn_pb, vshl_pb = state.pop(pb)
            ps_t = pst.tile([128, JL], F32d)
            nc.tensor.transpose(ps_t[:, :NT], pm1[:], id8[0:NT, 0:NT])
            ps_z = psqp.tile([1, 1], F32d, tag="psq")
            nc.tensor.matmul(ps_z[:], lhsT=ones8[:], rhs=zpart[:], start=True, stop=True)
            pT = smp.tile([128, NT], BF16d, tag="pT", bufs=2)
            nc.vector.tensor_copy(pT[:], ps_t[:, :NT])
            rs = smp.tile([1, 1], F32d, tag="rs", bufs=2)
            nc.vector.reciprocal(rs[:], ps_z[:])
            for dc in range(4):
                ps_o = pso.tile([1, 512], F32d)
                for t in range(NT):
                    nc.tensor.matmul(
                        ps_o[:], lhsT=pT[:, t : t + 1],
                        rhs=vn_pb[t][:, 512 * dc : 512 * dc + 512],
                        start=(t == 0), stop=False,
                    )
                nc.tensor.matmul(
                    ps_o[:], lhsT=ones2b[:],
                    rhs=vshl_pb[:, 512 * dc : 512 * dc + 512],
                    start=False, stop=True,
                )
                osl = outp.tile([1, 512], F32d, tag="osl", bufs=2, name=f"osl{pb}_{dc}")
                nc.vector.tensor_scalar_mul(osl[:], ps_o[:], rs[0:1, 0:1])
                nc.sync.dma_start(
                    out.ap()[pb : pb + 1, 512 * dc : 512 * dc + 512], osl[:]
                )

        vt_tiles = {}

        def load_vt(bb):
            """vt load split per k-pair so early main matmuls can start as
            soon as their pair lands (keeps batch 0 fed)."""
            if bb >= BL or bb in vt_tiles:
                return
            vt_t = vtp.tile(
                [128, KD, N], vdt, name=f"vt{bb}", bufs=2 if NT <= 6 else 1
            )
            for q in range(KD // 2):
                nc.sync.dma_start(
                    vt_t[:, 2 * q : 2 * q + 2, :], vt.ap()[bb, q, :, :, :]
                )
            vt_tiles[bb] = vt_t

        for b in range(BL):
            vn_b = None
            epi = None
            logit_sb = smp.tile([1, L], F32d, tag="logit", bufs=2 if NT <= 5 else 1)
            # tail [N, L) is never computed: weight must be exp(-1e-9) == 1.0f,
            # which equals exp(0), so zero-fill suffices.
            if N < L:
                nc.vector.memset(logit_sb[:, N:L], 0.0)
            mf_sb = smp.tile([1, N], F32d, tag="mf", bufs=2 if NT <= 5 else 1)
            nc.scalar.dma_start(mf_sb[:], mf.ap()[b : b + 1, :])
            if b == 0:
                fetch_wq(0)
                fetch_wv(0)
                fetch_wq(1)
                fetch_wv(1)
                load_vt(0)
                nc.scalar.dma_start(wo_sb[:], wo.ap()[:])
                nc.scalar.dma_start(bv_sb[:], bvt.ap()[:])
                nc.scalar.dma_start(bo_sb[:], bo.ap()[:])
            vt_b = vt_tiles.pop(b)
            for ci, (off, nl) in enumerate(chunks):
                if ci == len(chunks) - 1:
                    vn_b = []
                    state[b] = (logit_sb, vn_b, None)
                ps_l = psl.tile([1, 512], F32d)
                pending = []
                for j in range(KA):
                    if b == 0 and ci == 0:
                        fetch_wq(j + 2)
                        fetch_wv(j + 2)
                        qproj_group(j)
                    if ci == 1:
                        # natural-orientation values stream, interleaved into
                        # the second chunk in exact need-order on the sync
                        # ring (used by the epilogue inside batch b+1)
                        if j % 2 == 0 and j // 2 < NT:
                            t = j // 2
                            vtile = vnp.tile(
                                [128, D], BF16d, tag=f"vn{t}",
                                bufs=2 if NT <= 5 else 1,
                                name=f"vn{b}_{t}",
                            )
                            nc.sync.dma_start(vtile[:], vn.ap()[b, t, :, :])
                            vn_b.append(vtile)
                        if j == 11:
                            load_vt(b + 1)
                        if j == KA - 1:
                            vshl_b = vnp.tile(
                                [2, D], BF16d, tag="vshl",
                                bufs=2 if NT <= 5 else 1, name=f"vs{b}"
                            )
                            nc.sync.dma_start(vshl_b[:], vshl.ap()[b, :, :])
                            state[b] = (logit_sb, vn_b, vshl_b)
                    if b > 0 and ci == 0:
                        if j == EPI_A:
                            epi = epi_softmax(b - 1)
                        elif j == EPI_B:
                            epi_out(b - 1, *epi)
                    ps_m = psm.tile([128, 512], F32d)
                    if fp8:
                        for q in range(KD // 2):
                            nc.tensor.matmul(
                                ps_m[:, :nl],
                                lhsT=wv_sb[:, j, 2 * q : 2 * q + 2, :],
                                rhs=vt_b[:, 2 * q : 2 * q + 2, off : off + nl],
                                start=(q == 0),
                                stop=(q == KD // 2 - 1),
                                perf_mode=DR,
                            )
                    else:
                        for k in range(KD):
                            nc.tensor.matmul(
                                ps_m[:, :nl],
                                lhsT=wv_sb[:, j, k, :],
                                rhs=vt_b[:, k, off : off + nl],
                                start=(k == 0),
                                stop=(k == KD - 1),
                            )
                    s1 = s1p.tile([128, 512], F32d)
                    nc.scalar.activation(
                        s1[:, :nl], ps_m[:, :nl], AF.Lrelu,
                        bias=bv_sb[:, j : j + 1], scale=lr_scale, alpha=ALPHA,
                    )
                    th = thp.tile([128, 512], BF16d)
                    nc.scalar.activation(
                        th[:, :nl], s1[:, :nl], AF.Tanh, bias=qp_sb[:, j, b : b + 1]
                    )
                    # Wo matmuls trail the tanh producers by two j-groups so
                    # the PE rides out ACT bubbles (table swaps at batch
                    # boundaries) without stalling
                    if len(pending) == 2:
                        pj, pth = pending.pop(0)
                        nc.tensor.matmul(
                            ps_l[:, :nl], lhsT=wo_sb[:, pj : pj + 1], rhs=pth[:, :nl],
                            start=(pj == 0), stop=False,
                        )
                    pending.append((j, th))
                for pj, pth in pending:
                    nc.tensor.matmul(
                        ps_l[:, :nl], lhsT=wo_sb[:, pj : pj + 1], rhs=pth[:, :nl],
                        start=(pj == 0), stop=(pj == KA - 1),
                    )
                # logit lrelu: DVE mid-run (keeps the ACT queue short near
                # batch boundaries); ACT on the very last chunk (shortest
                # serial chain into the final epilogue)
                lsl = logit_sb[:, off : off + nl]
                if tail_warm and b == BL - 1 and ci == 1:
                    nc.scalar.activation(
                        lsl, ps_l[:, :nl], AF.Lrelu, bias=bo_sb[0:1, 0:1],
                        alpha=ALPHA,
                    )
                else:
                    lt1 = smp.tile([1, 512], F32d, tag="lt1", bufs=1)
                    nc.vector.tensor_scalar(
                        lt1[:, :nl], ps_l[:, :nl], bo_sb[0:1, 0:1], ALPHA,
                        mybir.AluOpType.add, mybir.AluOpType.mult,
                    )
                    nc.vector.tensor_scalar(
                        lsl, ps_l[:, :nl], bo_sb[0:1, 0:1], None,
                        mybir.AluOpType.add,
                    )
                    nc.vector.tensor_max(lsl, lsl, lt1[:, :nl])
                # mask: zero the logits of mask==0 / padded positions; their
                # weight exp(0) == 1.0f == exp(-1e-9), as the reference has it
                nc.vector.tensor_mul(lsl, lsl, mf_sb[:, off : off + nl])

        # last batch's epilogue runs serially after the loop. A dummy exp
        # pre-swaps the ACT table while the logit chain drains, and dummy
        # matmuls keep the PE clock ramped through the idle window so the
        # final out-GEMM runs at full p-state.
        if tail_warm:
            # pre-swap the ACT table to the exp-capable set while the logit
            # chain drains on DVE/DMA, so the real exp starts immediately
            dxp = smp.tile([1, 1], F32d, tag="dxp")
            nc.scalar.activation(dxp[:], bo_sb[0:1, 0:1], AF.Exp)
        epi = epi_softmax(BL - 1)
        if tail_warm:
            # a short keep-warm burst so the PE clock stays ramped through
            # the softmax chain and the final out-GEMM runs at full p-state
            wu2 = psqp.tile([128, 128], F32d, tag="psq")
            for i in range(24):
                nc.tensor.matmul(
                    wu2[:], lhsT=wu_l[:], rhs=wu_l[:], start=(i == 0), stop=(i == 23)
                )
        epi_out(BL - 1, *epi)

    nc.compile()
    return nc


def pad_n(max_n1):
    """Computed-position count: even split into two equal chunks, each a
    multiple of 2 and >= 128 so the PE never stalls on LDWEIGHTS."""
    half = max(128, (max_n1 + 1) // 2)
    half = (half + 1) // 2 * 2
    return min(2 * half, L)


def prep_inputs(query, values, mask, Wq, bq, Wv, bv, Wo, bo, mm="fp8"):
    """Host-side shard + layout prep. Returns (N, list of 8 in_maps)."""
    fp8 = mm == "fp8"
    mask = np.asarray(mask)
    n1s = (mask != 0).sum(axis=1)
    N = pad_n(int(n1s.max()))

    def wcast(w):
        if fp8:
            return (w * np.float32(W_SCALE)).astype(E4)
        return w.astype(BF)

    Wv32 = np.ascontiguousarray(Wv, np.float32)
    Wq32 = np.ascontiguousarray(Wq, np.float32)
    # wv[j, p, k, i] = Wv[128j+i, 128k+p]  (WvT, a-tile-major linear chunks)
    wv_t = wcast(
        np.ascontiguousarray(Wv32.reshape(KA, 128, KD, 128).transpose(0, 3, 2, 1))
    )
    # wq[t, p, k, i] = Wq[128t+i, 128k+p]  (WqT, a-tile-major linear chunks)
    wq_t = wcast(
        np.ascontiguousarray(Wq32.reshape(KA, 128, KD, 128).transpose(0, 3, 2, 1))
    )
    wo_t = np.ascontiguousarray(Wo.reshape(KA, 128).T).astype(BF)
    bv_t = np.ascontiguousarray(bv.reshape(KA, 128).T).astype(np.float32)
    bq_t = np.ascontiguousarray(bq.reshape(KA, 128).T).astype(np.float32)
    bo_r = np.asarray(bo, np.float32).reshape(1, 1)

    in_maps = []
    for i in range(NCORES):
        sl = slice(BL * i, BL * (i + 1))
        v = np.asarray(values[sl], np.float32)
        m = np.asarray(mask[sl])
        # permute each batch's L dim: mask!=0 positions first
        vp = np.empty_like(v)
        mp = np.empty_like(m)
        for bb in range(BL):
            perm = np.concatenate(
                [np.flatnonzero(m[bb] != 0), np.flatnonzero(m[bb] == 0)]
            )
            vp[bb] = v[bb, perm]
            mp[bb] = m[bb, perm]
        # vt[b, q, p, s, l] = vp[b, l, 128*(2q+s)+p] (k-pair-major linear runs)
        vt_i = np.ascontiguousarray(
            vp[:, :N, :]
            .reshape(BL, N, KD // 2, 2, 128)
            .transpose(0, 2, 4, 3, 1)
        )
        vt_i = vt_i.astype(E4) if fp8 else vt_i.astype(BF)
        # vn[b, t, p, d] = vp[b, 128t+p, d] for t < NT (gathered l-tiles)
        NT = (N + 127) // 128
        vn_i = np.ascontiguousarray(vp[:, : NT * 128].reshape(BL, NT, 128, D)).astype(
            BF
        )
        # sum over ALL positions, as a bf16 hi+lo pair (f32-accurate)
        vs = vp.sum(axis=1, dtype=np.float32)
        vs_hi = vs.astype(BF)
        vs_lo = (vs - vs_hi.astype(np.float32)).astype(BF)
        vshl_i = np.ascontiguousarray(np.stack([vs_hi, vs_lo], axis=1))
        # qt[p, k, b] = query[b, 128k+p]
        qt_i = np.ascontiguousarray(
            np.asarray(query[sl], np.float32).T.reshape(KD, 128, BL).transpose(1, 0, 2)
        )
        qt_i = qt_i.astype(E4) if fp8 else qt_i.astype(BF)
        mf_i = np.ascontiguousarray((mp[:, :N] != 0).astype(np.float32))
        in_maps.append(
            {
                "vt": vt_i, "vn": vn_i, "vshl": vshl_i, "wv": wv_t, "wq": wq_t,
                "qt": qt_i, "wo": wo_t, "bvt": bv_t, "bqt": bq_t, "bo": bo_r,
                "mf": mf_i, "id4": np.eye(JL, dtype=np.float32),
            }
        )
    return N, in_maps


_NC_CACHE = {}


def get_graph(N, mm="fp8", tail_warm=False):
    key = (N, mm, tail_warm)
    if key not in _NC_CACHE:
        _NC_CACHE[key] = build_graph(N, mm, tail_warm)
    return _NC_CACHE[key]


def run(inputs, trace=False, mm="fp8", tail_warm=False):
    N, in_maps = prep_inputs(**inputs, mm=mm)
    nc = get_graph(N, mm, tail_warm)
    res = bass_utils.run_bass_kernel_spmd(
        nc, in_maps, core_ids=list(range(NCORES)), trace=trace
    )
    out = np.concatenate([res.results[i]["out"] for i in range(NCORES)], axis=0)
    return out.astype(np.float32), res


def kernel(**inputs):
    out, _ = run(inputs, trace=False)
    return out


# revision 61
# speedup vs baseline: 1.2030x; 1.2030x over previous
"""Trainium2 Bass kernel for the masked-softmax attention module.

Computation (per batch row b):
    m      = lrelu(values[b] @ Wv.T + bv) + lrelu(query[b] @ Wq.T + bq)   [L, A]
    logit  = lrelu(tanh(m) @ Wo.T + bo)                                    [L]
    logit  = where(mask[b] == 0, -1e-9, logit)
    prob   = softmax(logit)
    out[b] = prob @ values[b]                                              [D]

Sparsity: positions with mask==0 get logit = -1e-9, so their softmax
weight is exactly exp(-1e-9) == 1.0f regardless of the expensive
pipeline. Host-side we PERMUTE each batch's L dim so mask==1 positions
come first (n1 of them), and only compute the m/tanh/Wo pipeline for
the first N >= max_b(n1) positions. Logits at [n1, N) are masked to
-1e-9 (exactly as the reference masks them) and [N, L) are memset to 0
(exp(0) == exp(-1e-9) == 1.0f). The softmax + out GEMM then run over
the full permuted L — numerically identical to the dense reference.

Main GEMM and q-projection run in fp8 (e4m3): values/query cast
directly (absmax ~5.4 << 240), Wv/Wq pre-scaled by 2^10 so their
entries are normal-range; the 2^-10 unscale is folded into the lrelu
ACT's scale input (exact, power of two). DoubleRow perf mode processes
2 k-tiles per matmul.

The per-batch softmax epilogue (exp/recip/transpose/out-GEMM) is
software-pipelined into the NEXT batch's first chunk so the in-order
PE never stalls on the softmax chain at batch boundaries.

Sharding: data-parallel over batch, 4 batches per core on 8 NeuronCores.
"""

import os
import sys

if "/opt/trn_rl_repo" not in sys.path:
    sys.path.insert(0, "/opt/trn_rl_repo")

import numpy as np
import ml_dtypes

from contextlib import ExitStack

import concourse.bass as bass
import concourse.tile as tile
from concourse import bacc, mybir
from concourse import bass_utils

BF = ml_dtypes.bfloat16
E4 = ml_dtypes.float8_e4m3
F32d = mybir.dt.float32
BF16d = mybir.dt.bfloat16
FP8d = mybir.dt.float8e4
AF = mybir.ActivationFunctionType
DR = mybir.MatmulPerfMode.DoubleRow

NCORES = 8
B, L, D, A = 32, 1024, 2048, 2048
BL = B // NCORES          # batches per core
KD = D // 128             # d tiles
KA = A // 128             # a tiles
JL = L // 128             # l tiles
ALPHA = 0.01              # leaky relu slope
W_SCALE = 1024.0          # host premultiplier on Wv/Wq for fp8 dynamic range
EPI_A = 1                 # j-slot where prev batch's softmax ops are emitted
EPI_B = 6                 # j-slot where prev batch's transpose+out GEMM goes


def build_graph(N, mm="fp8", tail_warm=False):
    """Build the per-core Bass graph (identical on all cores).

    N: padded count of computed positions per batch (even).
    mm: "fp8" (DoubleRow e4m3 main GEMM + qproj) or "bf16".
    tail_warm: keep-warm dummies + ACT-table pre-swap before the last
    epilogue.
    """
    nc = bacc.Bacc("TRN2", target_bir_lowering=False, debug=False)
    NL1 = N // 2
    chunks = [(0, NL1), (NL1, NL1)]
    fp8 = mm == "fp8"
    vdt = FP8d if fp8 else BF16d

    NT = (N + 127) // 128  # l-tiles needed by the gathered out-GEMM

    # layouts are chosen so every DMA slice is one LINEAR run in dram
    # (scattered per-partition rows run ~4x slower on the DGE rings)
    vt = nc.dram_tensor("vt", [BL, KD // 2, 128, 2, N], vdt, kind="ExternalInput")
    vn = nc.dram_tensor("vn", [BL, NT, 128, D], BF16d, kind="ExternalInput")
    vshl = nc.dram_tensor("vshl", [BL, 2, D], BF16d, kind="ExternalInput")
    wv = nc.dram_tensor("wv", [KA, 128, KD, 128], vdt, kind="ExternalInput")
    wq = nc.dram_tensor("wq", [KA, 128, KD, 128], vdt, kind="ExternalInput")
    qt = nc.dram_tensor("qt", [128, KD, BL], vdt, kind="ExternalInput")
    wo = nc.dram_tensor("wo", [128, KA], BF16d, kind="ExternalInput")
    bvt = nc.dram_tensor("bvt", [128, KA], F32d, kind="ExternalInput")
    bqt = nc.dram_tensor("bqt", [128, KA], F32d, kind="ExternalInput")
    bo = nc.dram_tensor("bo", [1, 1], F32d, kind="ExternalInput")
    mf = nc.dram_tensor("mf", [BL, N], F32d, kind="ExternalInput")
    id4d = nc.dram_tensor("id4", [JL, JL], F32d, kind="ExternalInput")
    out = nc.dram_tensor("out", [BL, D], F32d, kind="ExternalOutput")

    lr_scale = 1.0 / W_SCALE if fp8 else 1.0

    with tile.TileContext(nc) as tc, ExitStack() as ctx:
        const = ctx.enter_context(tc.tile_pool(name="const", bufs=1))
        wvp = ctx.enter_context(tc.tile_pool(name="wvp", bufs=1))

        # One FIFO HWDGE ring (sync) carries all bulk loads in exact
        # need-order (the 16 shared DMA engines serve one queue at a time,
        # so a bulk transfer on another ring would starve the weight
        # stream); the scalar ring carries only small constants.
        qts_sb = const.tile([128, KD, BL], vdt)
        nc.sync.dma_start(qts_sb[:], qt.ap()[:])
        bq_sb = const.tile([128, KA], F32d)
        nc.sync.dma_start(bq_sb[:], bqt.ap()[:])
        id8 = const.tile([JL, JL], F32d)
        nc.scalar.dma_start(id8[:], id4d.ap()[:])
        ones8 = const.tile([JL, 1], F32d)
        nc.vector.memset(ones8[:], 1.0)
        ones2b = const.tile([2, 1], BF16d)
        nc.vector.memset(ones2b[:], 1.0)
        qp_sb = const.tile([128, KA, BL], F32d)
        # allocated here, DMA'd after the first vt chunk (see main loop)
        wo_sb = const.tile([128, KA], BF16d)
        bv_sb = const.tile([128, KA], F32d)
        bo_sb = const.tile([1, 1], F32d)

        # wv is laid out a-tile-major: GEMM group j only needs its own chunk,
        # so chunks stream just-in-time, interleaved with wq below.
        wv_sb = wvp.tile([128, KA, KD, 128], vdt)
        wv_loaded = set()

        def fetch_wv(j):
            if j < KA and j not in wv_loaded:
                nc.sync.dma_start(wv_sb[:, j, :, :], wv.ap()[j, :, :, :])
                wv_loaded.add(j)

        # q-projection is interleaved into batch 0 / chunk 0 of the main loop
        # (one group per GEMM group) so its wq DMA demand spreads out and the
        # PE never sits idle waiting for the projection phase.
        wqp = ctx.enter_context(tc.tile_pool(name="wqp", bufs=6 if NT <= 5 else 4))
        psqp = ctx.enter_context(tc.tile_pool(name="psq", bufs=1, space="PSUM"))
        wq_tiles = {}

        def fetch_wq(t):
            if t < KA and t not in wq_tiles:
                wq_t = wqp.tile([128, KD, 128], vdt)
                nc.sync.dma_start(wq_t[:], wq.ap()[t, :, :, :])
                wq_tiles[t] = wq_t

        def qproj_group(t):
            wq_t = wq_tiles.pop(t)
            psq = psqp.tile([128, BL], F32d)
            if fp8:
                for q in range(KD // 2):
                    nc.tensor.matmul(
                        psq[:], lhsT=wq_t[:, 2 * q : 2 * q + 2, :],
                        rhs=qts_sb[:, 2 * q : 2 * q + 2, :],
                        start=(q == 0), stop=(q == KD // 2 - 1), perf_mode=DR,
                    )
            else:
                for k in range(KD):
                    nc.tensor.matmul(
                        psq[:], lhsT=wq_t[:, k, :], rhs=qts_sb[:, k, :],
                        start=(k == 0), stop=(k == KD - 1),
                    )
            nc.scalar.activation(
                qp_sb[:, t, :], psq[:], AF.Lrelu, bias=bq_sb[:, t : t + 1],
                scale=lr_scale, alpha=ALPHA,
            )

        # PE warmup: dummy matmuls on zeroed tiles while the first DMAs land,
        # so the HAM clock gate is released before real work starts.
        wu_l = const.tile([128, 128], BF16d)
        nc.vector.memset(wu_l[:], 0.0)
        wu_ps = psqp.tile([128, 128], F32d, tag="psq")
        for i in range(32):
            nc.tensor.matmul(wu_ps[:], lhsT=wu_l[:], rhs=wu_l[:], start=(i == 0), stop=(i == 31))

        # ---- main loop ----
        vtp = ctx.enter_context(tc.tile_pool(name="vtp", bufs=2))
        vnp = ctx.enter_context(tc.tile_pool(name="vnp", bufs=2))
        s1p = ctx.enter_context(tc.tile_pool(name="s1p", bufs=2))
        thp = ctx.enter_context(tc.tile_pool(name="thp", bufs=4))
        smp = ctx.enter_context(tc.tile_pool(name="smp", bufs=1))
        outp = ctx.enter_context(tc.tile_pool(name="outp", bufs=1))
        # 8 PSUM banks total: psm 4 (main-GEMM runway against ACT bubbles),
        # psl 1, psq 1 (warmup/qproj/transpose/Z share one slot serially),
        # pso 2
        psm = ctx.enter_context(tc.tile_pool(name="psm", bufs=4, space="PSUM"))
        psl = ctx.enter_context(tc.tile_pool(name="psl", bufs=1, space="PSUM"))
        pso = ctx.enter_context(tc.tile_pool(name="pso", bufs=2, space="PSUM"))

        state = {}  # b -> (logit_sb, [vn tiles], vshl tile)

        def epi_softmax(pb):
            """Softmax chain for batch pb (ACT/DVE/DMA engines only).

            Reshape logits [1, L] -> [JL, 128] first (tiny DMA), then exp on
            8 partitions (8x faster than a 1-partition exp) with per-
            partition accumulators; Z is reduced by a tiny ones-matmul.

            The out GEMM uses the identity (gathered form):
              out*Z = sum_{i<NT*128} (e^{logit_i} - 1) v_i + sum_all v
            (positions >= n1 have logit 0 -> weight 0), so only NT l-tiles
            of values are streamed; sum_all v enters as a bf16 hi+lo pair."""
            logit_pb = state[pb][0]
            l8 = smp.tile([JL, 128], F32d, tag="l8", bufs=2)
            nc.sync.dma_start(l8[:], logit_pb[:])
            p8 = smp.tile([JL, 128], F32d, tag="p8", bufs=2)
            zpart = smp.tile([JL, 1], F32d, tag="zp", bufs=2)
            nc.scalar.activation(p8[:], l8[:], AF.Exp, accum_out=zpart[:])
            pm1 = smp.tile([NT, 128], F32d, tag="pm1", bufs=2)
            nc.vector.tensor_scalar(
                pm1[:], p8[0:NT, :], -1.0, None, mybir.AluOpType.add
            )
            return zpart, pm1

        def epi_out(pb, zpart, pm1):
            """Transpose + out GEMM for batch pb (PE heavy)."""
            _, vn_pb, vshl_pb = state.pop(pb)
            # transpose scratch and Z share the psq bank serially: the Z
            # matmul's slot reuse waits for the pT copy, which is emitted
            # first so the WAR dependency is visible to the scheduler
            ps_t = psqp.tile([128, JL], F32d, tag="psq", name=f"ps_t{pb}")
            nc.tensor.transpose(ps_t[:, :NT], pm1[:], id8[0:NT, 0:NT])
            pT = smp.tile([128, NT], BF16d, tag="pT", bufs=2)
            nc.vector.tensor_copy(pT[:], ps_t[:, :NT])
            ps_z = psqp.tile([1, 1], F32d, tag="psq", name=f"ps_z{pb}")
            nc.tensor.matmul(ps_z[:], lhsT=ones8[:], rhs=zpart[:], start=True, stop=True)
            rs = smp.tile([1, 1], F32d, tag="rs", bufs=2)
            nc.vector.reciprocal(rs[:], ps_z[:])
            for dc in range(4):
                ps_o = pso.tile([1, 512], F32d)
                for t in range(NT):
                    nc.tensor.matmul(
                        ps_o[:], lhsT=pT[:, t : t + 1],
                        rhs=vn_pb[t][:, 512 * dc : 512 * dc + 512],
                        start=(t == 0), stop=False,
                    )
                nc.tensor.matmul(
                    ps_o[:], lhsT=ones2b[:],
                    rhs=vshl_pb[:, 512 * dc : 512 * dc + 512],
                    start=False, stop=True,
                )
                osl = outp.tile([1, 512], F32d, tag="osl", bufs=2, name=f"osl{pb}_{dc}")
                nc.vector.tensor_scalar_mul(osl[:], ps_o[:], rs[0:1, 0:1])
                nc.sync.dma_start(
                    out.ap()[pb : pb + 1, 512 * dc : 512 * dc + 512], osl[:]
                )

        vt_tiles = {}

        def load_vt(bb):
            """vt load split per k-pair so early main matmuls can start as
            soon as their pair lands (keeps batch 0 fed)."""
            if bb >= BL or bb in vt_tiles:
                return
            vt_t = vtp.tile(
                [128, KD, N], vdt, name=f"vt{bb}", bufs=2 if NT <= 6 else 1
            )
            for q in range(KD // 2):
                nc.sync.dma_start(
                    vt_t[:, 2 * q : 2 * q + 2, :], vt.ap()[bb, q, :, :, :]
                )
            vt_tiles[bb] = vt_t

        for b in range(BL):
            vn_b = None
            epi = None
            logit_sb = smp.tile([1, L], F32d, tag="logit", bufs=2 if NT <= 5 else 1)
            # tail [N, L) is never computed: weight must be exp(-1e-9) == 1.0f,
            # which equals exp(0), so zero-fill suffices.
            if N < L:
                nc.vector.memset(logit_sb[:, N:L], 0.0)
            mf_sb = smp.tile([1, N], F32d, tag="mf", bufs=2 if NT <= 5 else 1)
            nc.scalar.dma_start(mf_sb[:], mf.ap()[b : b + 1, :])
            if b == 0:
                fetch_wq(0)
                fetch_wv(0)
                fetch_wq(1)
                fetch_wv(1)
                load_vt(0)
                nc.scalar.dma_start(wo_sb[:], wo.ap()[:])
                nc.scalar.dma_start(bv_sb[:], bvt.ap()[:])
                nc.scalar.dma_start(bo_sb[:], bo.ap()[:])
            vt_b = vt_tiles.pop(b)
            for ci, (off, nl) in enumerate(chunks):
                if ci == len(chunks) - 1:
                    vn_b = []
                    state[b] = (logit_sb, vn_b, None)
                ps_l = psl.tile([1, 512], F32d)
                pending = []
                for j in range(KA):
                    if b == 0 and ci == 0:
                        fetch_wq(j + 2)
                        fetch_wv(j + 2)
                        qproj_group(j)
                    if ci == 1:
                        # natural-orientation values stream, interleaved into
                        # the second chunk in exact need-order on the sync
                        # ring (used by the epilogue inside batch b+1)
                        if j % 2 == 0 and j // 2 < NT:
                            t = j // 2
                            vtile = vnp.tile(
                                [128, D], BF16d, tag=f"vn{t}",
                                bufs=2 if NT <= 5 else 1,
                                name=f"vn{b}_{t}",
                            )
                            nc.sync.dma_start(vtile[:], vn.ap()[b, t, :, :])
                            vn_b.append(vtile)
                        if j == 11:
                            load_vt(b + 1)
                        if j == KA - 1:
                            vshl_b = vnp.tile(
                                [2, D], BF16d, tag="vshl",
                                bufs=2 if NT <= 5 else 1, name=f"vs{b}"
                            )
                            nc.sync.dma_start(vshl_b[:], vshl.ap()[b, :, :])
                            state[b] = (logit_sb, vn_b, vshl_b)
                    if b > 0 and ci == 0:
                        if j == EPI_A:
                            epi = epi_softmax(b - 1)
                        elif j == EPI_B:
                            epi_out(b - 1, *epi)
                    ps_m = psm.tile([128, 512], F32d)
                    if fp8:
                        for q in range(KD // 2):
                            nc.tensor.matmul(
                                ps_m[:, :nl],
                                lhsT=wv_sb[:, j, 2 * q : 2 * q + 2, :],
                                rhs=vt_b[:, 2 * q : 2 * q + 2, off : off + nl],
                                start=(q == 0),
                                stop=(q == KD // 2 - 1),
                                perf_mode=DR,
                            )
                    else:
                        for k in range(KD):
                            nc.tensor.matmul(
                                ps_m[:, :nl],
                                lhsT=wv_sb[:, j, k, :],
                                rhs=vt_b[:, k, off : off + nl],
                                start=(k == 0),
                                stop=(k == KD - 1),
                            )
                    s1 = s1p.tile([128, 512], F32d)
                    nc.scalar.activation(
                        s1[:, :nl], ps_m[:, :nl], AF.Lrelu,
                        bias=bv_sb[:, j : j + 1], scale=lr_scale, alpha=ALPHA,
                    )
                    th = thp.tile([128, 512], BF16d)
                    nc.scalar.activation(
                        th[:, :nl], s1[:, :nl], AF.Tanh, bias=qp_sb[:, j, b : b + 1]
                    )
                    # Wo matmuls trail the tanh producers by two j-groups so
                    # the PE rides out ACT bubbles (table swaps at batch
                    # boundaries) without stalling
                    if len(pending) == 2:
                        pj, pth = pending.pop(0)
                        nc.tensor.matmul(
                            ps_l[:, :nl], lhsT=wo_sb[:, pj : pj + 1], rhs=pth[:, :nl],
                            start=(pj == 0), stop=False,
                        )
                    pending.append((j, th))
                for pj, pth in pending:
                    nc.tensor.matmul(
                        ps_l[:, :nl], lhsT=wo_sb[:, pj : pj + 1], rhs=pth[:, :nl],
                        start=(pj == 0), stop=(pj == KA - 1),
                    )
                # logit lrelu: DVE mid-run (keeps the ACT queue short near
                # batch boundaries); ACT on the very last chunk (shortest
                # serial chain into the final epilogue)
                lsl = logit_sb[:, off : off + nl]
                if tail_warm and b == BL - 1 and ci == 1:
                    nc.scalar.activation(
                        lsl, ps_l[:, :nl], AF.Lrelu, bias=bo_sb[0:1, 0:1],
                        alpha=ALPHA,
                    )
                else:
                    lt1 = smp.tile([1, 512], F32d, tag="lt1", bufs=1)
                    nc.vector.tensor_scalar(
                        lt1[:, :nl], ps_l[:, :nl], bo_sb[0:1, 0:1], ALPHA,
                        mybir.AluOpType.add, mybir.AluOpType.mult,
                    )
                    nc.vector.tensor_scalar(
                        lsl, ps_l[:, :nl], bo_sb[0:1, 0:1], None,
                        mybir.AluOpType.add,
                    )
                    nc.vector.tensor_max(lsl, lsl, lt1[:, :nl])
                # mask: zero the logits of mask==0 / padded positions; their
                # weight exp(0) == 1.0f == exp(-1e-9), as the reference has it
                nc.vector.tensor_mul(lsl, lsl, mf_sb[:, off : off + nl])

        # last batch's epilogue runs serially after the loop. A dummy exp
        # pre-swaps the ACT table while the logit chain drains, and dummy
        # matmuls keep the PE clock ramped through the idle window so the
        # final out-GEMM runs at full p-state.
        if tail_warm:
            # pre-swap the ACT table to the exp-capable set while the logit
            # chain drains on DVE/DMA, so the real exp starts immediately
            dxp = smp.tile([1, 1], F32d, tag="dxp")
            nc.scalar.activation(dxp[:], bo_sb[0:1, 0:1], AF.Exp)
        epi = epi_softmax(BL - 1)
        if tail_warm:
            # a short keep-warm burst so the PE clock stays ramped through
            # the softmax chain and the final out-GEMM runs at full p-state
            wu2 = psqp.tile([128, 128], F32d, tag="psq")
            for i in range(24):
                nc.tensor.matmul(
                    wu2[:], lhsT=wu_l[:], rhs=wu_l[:], start=(i == 0), stop=(i == 23)
                )
        epi_out(BL - 1, *epi)

    nc.compile()
    return nc


def pad_n(max_n1):
    """Computed-position count: even split into two equal chunks, each a
    multiple of 2 and >= 128 so the PE never stalls on LDWEIGHTS."""
    half = max(128, (max_n1 + 1) // 2)
    half = (half + 1) // 2 * 2
    return min(2 * half, L)


def prep_inputs(query, values, mask, Wq, bq, Wv, bv, Wo, bo, mm="fp8"):
    """Host-side shard + layout prep. Returns (N, list of 8 in_maps)."""
    fp8 = mm == "fp8"
    mask = np.asarray(mask)
    n1s = (mask != 0).sum(axis=1)
    N = pad_n(int(n1s.max()))

    def wcast(w):
        if fp8:
            return (w * np.float32(W_SCALE)).astype(E4)
        return w.astype(BF)

    Wv32 = np.ascontiguousarray(Wv, np.float32)
    Wq32 = np.ascontiguousarray(Wq, np.float32)
    # wv[j, p, k, i] = Wv[128j+i, 128k+p]  (WvT, a-tile-major linear chunks)
    wv_t = wcast(
        np.ascontiguousarray(Wv32.reshape(KA, 128, KD, 128).transpose(0, 3, 2, 1))
    )
    # wq[t, p, k, i] = Wq[128t+i, 128k+p]  (WqT, a-tile-major linear chunks)
    wq_t = wcast(
        np.ascontiguousarray(Wq32.reshape(KA, 128, KD, 128).transpose(0, 3, 2, 1))
    )
    wo_t = np.ascontiguousarray(Wo.reshape(KA, 128).T).astype(BF)
    bv_t = np.ascontiguousarray(bv.reshape(KA, 128).T).astype(np.float32)
    bq_t = np.ascontiguousarray(bq.reshape(KA, 128).T).astype(np.float32)
    bo_r = np.asarray(bo, np.float32).reshape(1, 1)

    in_maps = []
    for i in range(NCORES):
        sl = slice(BL * i, BL * (i + 1))
        v = np.asarray(values[sl], np.float32)
        m = np.asarray(mask[sl])
        # permute each batch's L dim: mask!=0 positions first
        vp = np.empty_like(v)
        mp = np.empty_like(m)
        for bb in range(BL):
            perm = np.concatenate(
                [np.flatnonzero(m[bb] != 0), np.flatnonzero(m[bb] == 0)]
            )
            vp[bb] = v[bb, perm]
            mp[bb] = m[bb, perm]
        # vt[b, q, p, s, l] = vp[b, l, 128*(2q+s)+p] (k-pair-major linear runs)
        vt_i = np.ascontiguousarray(
            vp[:, :N, :]
            .reshape(BL, N, KD // 2, 2, 128)
            .transpose(0, 2, 4, 3, 1)
        )
        vt_i = vt_i.astype(E4) if fp8 else vt_i.astype(BF)
        # vn[b, t, p, d] = vp[b, 128t+p, d] for t < NT (gathered l-tiles)
        NT = (N + 127) // 128
        vn_i = np.ascontiguousarray(vp[:, : NT * 128].reshape(BL, NT, 128, D)).astype(
            BF
        )
        # sum over ALL positions, as a bf16 hi+lo pair (f32-accurate)
        vs = vp.sum(axis=1, dtype=np.float32)
        vs_hi = vs.astype(BF)
        vs_lo = (vs - vs_hi.astype(np.float32)).astype(BF)
        vshl_i = np.ascontiguousarray(np.stack([vs_hi, vs_lo], axis=1))
        # qt[p, k, b] = query[b, 128k+p]
        qt_i = np.ascontiguousarray(
            np.asarray(query[sl], np.float32).T.reshape(KD, 128, BL).transpose(1, 0, 2)
        )
        qt_i = qt_i.astype(E4) if fp8 else qt_i.astype(BF)
        mf_i = np.ascontiguousarray((mp[:, :N] != 0).astype(np.float32))
        in_maps.append(
            {
                "vt": vt_i, "vn": vn_i, "vshl": vshl_i, "wv": wv_t, "wq": wq_t,
                "qt": qt_i, "wo": wo_t, "bvt": bv_t, "bqt": bq_t, "bo": bo_r,
                "mf": mf_i, "id4": np.eye(JL, dtype=np.float32),
            }
        )
    return N, in_maps


_NC_CACHE = {}


def get_graph(N, mm="fp8", tail_warm=False):
    key = (N, mm, tail_warm)
    if key not in _NC_CACHE:
        _NC_CACHE[key] = build_graph(N, mm, tail_warm)
    return _NC_CACHE[key]


def run(inputs, trace=False, mm="fp8", tail_warm=False):
    N, in_maps = prep_inputs(**inputs, mm=mm)
    nc = get_graph(N, mm, tail_warm)
    res = bass_utils.run_bass_kernel_spmd(
        nc, in_maps, core_ids=list(range(NCORES)), trace=trace
    )
    out = np.concatenate([res.results[i]["out"] for i in range(NCORES)], axis=0)
    return out.astype(np.float32), res


def kernel(**inputs):
    out, _ = run(inputs, trace=False)
    return out
